# revision 9
# baseline (speedup 1.0000x reference)
"""ColBERT pairwise-distill loss on 8 Trainium2 NeuronCores (Bass/Tile).

Strategy (doc-shard): each core holds all queries plus a 1/8 shard of the
docs (8 docs x 1024 tokens), computes local MaxSim scores [64, 8] for the
student and teacher streams, all-gathers score columns across the 8 cores,
and every core computes the final scalar loss (output read from core 0).

Per-core compute pipeline, per (tensor, query-chunk) unit:
  - 16 bf16 matmuls (K=128, M=128 q-tokens, N=512 doc-tokens) fill PSUM
    with fp32 similarities, 2 docs (4 banks) per wave.
  - wave 0 (docs 0-1): DVE reduce_max directly from PSUM (fp32).
  - waves 1-3 (docs 2-7): ScalarE evacuates PSUM -> SBUF bf16; VectorE
    folds a pairwise max tree at 2x bf16 rate, then a small tail reduce.
  - query tokens are laid out n-major so the sum over the 32 query tokens
    becomes two tiny strided reduce_sums plus one add.
"""

import numpy as np
import ml_dtypes

_BF16 = ml_dtypes.bfloat16

B, N, D, S, C = 64, 32, 128, 1024, 64
NCORES = 8
CL = C // NCORES          # docs per core
QCH = (B * N) // 128      # query-token chunks of 128
A_DOCS = 2                # docs per unit reduced directly from PSUM (fp32)
B_DOCS = CL - A_DOCS      # docs per unit via ACT-evac + bf16 max tree

_CACHE = {}


def _build_nc():
    import concourse.tile as tile
    from concourse import bacc, mybir

    f32 = mybir.dt.float32
    bf16 = mybir.dt.bfloat16
    f16 = mybir.dt.float16
    X = mybir.AxisListType.X
    MAX = mybir.AluOpType.max
    ADD = mybir.AluOpType.add
    SUB = mybir.AluOpType.subtract
    MULT = mybir.AluOpType.mult

    nc = bacc.Bacc(
        "TRN2", target_bir_lowering=False, debug=False, num_devices=NCORES
    )
    qT_d = nc.dram_tensor("qT", [128, B * N], bf16, kind="ExternalInput")
    tqT_d = nc.dram_tensor("tqT", [128, B * N], bf16, kind="ExternalInput")
    dT_d = nc.dram_tensor("dT", [128, CL * S], bf16, kind="ExternalInput")
    tdT_d = nc.dram_tensor("tdT", [128, CL * S], bf16, kind="ExternalInput")
    eye_d = nc.dram_tensor("eye", [B, B], f32, kind="ExternalInput")
    neye_d = nc.dram_tensor("neye", [B, B], f32, kind="ExternalInput")
    loss_d = nc.dram_tensor("loss", [1, 1], f32, kind="ExternalOutput")

    with tile.TileContext(nc) as tc:
        with (
            tc.tile_pool(name="inp", bufs=1) as inp,
            tc.tile_pool(name="ps", bufs=2, space="PSUM") as psp,
            tc.tile_pool(name="slab", bufs=3) as slabp,
            tc.tile_pool(name="tree", bufs=2) as treep,
            tc.tile_pool(name="fin", bufs=1) as fin,
            tc.tile_pool(name="dram", bufs=1, space="DRAM") as drp,
        ):
            qTs = inp.tile([128, B * N], bf16, tag="qT")
            tqTs = inp.tile([128, B * N], bf16, tag="tqT")
            dTs = inp.tile([128, CL * S], bf16, tag="dT")
            tdTs = inp.tile([128, CL * S], bf16, tag="tdT")
            nc.sync.dma_start(out=qTs, in_=qT_d.ap())
            nc.sync.dma_start(out=tqTs, in_=tqT_d.ap())
            nc.sync.dma_start(out=dTs, in_=dT_d.ap())
            nc.sync.dma_start(out=tdTs, in_=tdT_d.ap())

            eye_s = fin.tile([B, B], f32, tag="eye")
            neye_s = fin.tile([B, B], f32, tag="neye")
            nc.sync.dma_start(out=eye_s, in_=eye_d.ap())
            nc.sync.dma_start(out=neye_s, in_=neye_d.ap())

            # Load the exp/ln activation tables up front so they do not
            # stall the loss stage at the end.
            warm = fin.tile([64, 1], f32, tag="warm")
            nc.vector.memset(warm, 1.0)
            nc.scalar.activation(
                out=warm, in_=warm, func=mybir.ActivationFunctionType.Exp
            )
            nc.scalar.activation(
                out=warm, in_=warm, func=mybir.ActivationFunctionType.Ln
            )

            # buf[t][p, j, c]: per-chunk per-doc max-sums (fp32)
            buf = [
                fin.tile([128, QCH, CL], f32, name=f"buf{t}", tag=f"buf{t}")
                for t in range(2)
            ]

            for t, (qq, dd_) in enumerate([(qTs, dTs), (tqTs, tdTs)]):
                for j in range(QCH):
                    lhsT = qq[:, j * 128 : (j + 1) * 128]
                    slab = slabp.tile([128, B_DOCS, S], bf16, tag="slab")
                    for w in range(CL // 2):
                        pt = psp.tile([128, 2, S], f32, tag="pt")
                        for dd in range(2):
                            base = (w * 2 + dd) * S
                            for h in range(2):
                                nc.tensor.matmul(
                                    pt[:, dd, h * 512 : (h + 1) * 512],
                                    lhsT,
                                    dd_[:, base + h * 512 : base + (h + 1) * 512],
                                    start=True,
                                    stop=True,
                                )
                        if w == 0:
                            nc.vector.reduce_max(
                                buf[t][:, j, 0:A_DOCS], pt, axis=X
                            )
                        else:
                            nc.scalar.copy(
                                out=slab[:, (w - 1) * 2 : (w - 1) * 2 + 2, :],
                                in_=pt,
                            )
                    # bf16 pairwise max tree: 1024 -> 64, then tail reduce
                    t1 = treep.tile([128, B_DOCS, 512], bf16, tag="t1")
                    nc.vector.tensor_tensor(
                        out=t1, in0=slab[:, :, 0:512], in1=slab[:, :, 512:1024],
                        op=MAX,
                    )
                    t2 = treep.tile([128, B_DOCS, 256], bf16, tag="t2")
                    nc.vector.tensor_tensor(
                        out=t2, in0=t1[:, :, 0:256], in1=t1[:, :, 256:512], op=MAX
                    )
                    t3 = treep.tile([128, B_DOCS, 128], bf16, tag="t3")
                    nc.vector.tensor_tensor(
                        out=t3, in0=t2[:, :, 0:128], in1=t2[:, :, 128:256], op=MAX
                    )
                    t4 = treep.tile([128, B_DOCS, 64], bf16, tag="t4")
                    nc.vector.tensor_tensor(
                        out=t4, in0=t3[:, :, 0:64], in1=t3[:, :, 64:128], op=MAX
                    )
                    nc.vector.reduce_max(buf[t][:, j, A_DOCS:CL], t4, axis=X)

            # Sum over query tokens: chunk j holds tokens (n = 2j + p//64,
            # b = p%64), so sum over j (strided reduce) and the two halves.
            sc = fin.tile([64, 2, CL], f32, tag="sc")
            for t in range(2):
                h0 = fin.tile([64, CL], f32, name=f"h0_{t}", tag=f"h0_{t}")
                h1 = fin.tile([64, CL], f32, name=f"h1_{t}", tag=f"h1_{t}")
                nc.vector.reduce_sum(
                    h0, buf[t][0:64].rearrange("p j c -> p c j"), axis=X
                )
                nc.vector.reduce_sum(
                    h1, buf[t][64:128].rearrange("p j c -> p c j"), axis=X
                )
                nc.vector.tensor_tensor(out=sc[:, t, :], in0=h0, in1=h1, op=ADD)

            # All-gather local [64, 2, CL] score columns across the 8 cores.
            sc_bounce = drp.tile([2, 64, CL], f32, tag="scb")
            nc.sync.dma_start(
                out=sc_bounce.rearrange("t b c -> b t c"), in_=sc
            )
            gat = drp.tile([NCORES, 2, 64, CL], f32, tag="gat")
            nc.gpsimd.collective_compute(
                "AllGather",
                mybir.AluOpType.bypass,
                replica_groups=[list(range(NCORES))],
                ins=[sc_bounce[:, :, :]],
                outs=[gat[:, :, :, :]],
            )
            scf = fin.tile([64, 2, NCORES * CL], f32, tag="scf")
            for t in range(2):
                nc.sync.dma_start(
                    out=scf[:, t, :].rearrange("b (r c) -> b r c", r=NCORES),
                    in_=gat[:, t, :, :].rearrange("r b c -> b r c"),
                )
            s_s = scf[:, 0, :]
            s_t = scf[:, 1, :]

            # Contrastive term.
            tmp = fin.tile([64, B], f32, tag="tmp")
            pos = fin.tile([64, 1], f32, tag="pos")
            neg = fin.tile([64, 1], f32, tag="neg")
            x = fin.tile([64, 1], f32, tag="x")
            ax = fin.tile([64, 1], f32, tag="ax")
            e = fin.tile([64, 1], f32, tag="e")
            l1p = fin.tile([64, 1], f32, tag="l1p")
            relux = fin.tile([64, 1], f32, tag="relux")
            pk = fin.tile([64, 2], f32, tag="pk")

            nc.vector.tensor_tensor(out=tmp, in0=s_s, in1=eye_s, op=MULT)
            nc.vector.reduce_sum(pos, tmp, axis=X)
            nc.vector.tensor_tensor(out=tmp, in0=s_s, in1=neye_s, op=ADD)
            nc.vector.reduce_max(neg, tmp, axis=X)
            nc.vector.tensor_tensor(out=x, in0=neg, in1=pos, op=SUB)
            # softplus(x) = max(x, 0) + ln(1 + exp(-|x|))
            nc.vector.scalar_tensor_tensor(
                out=ax, in0=x, scalar=-1.0, in1=x, op0=MULT, op1=MAX
            )
            nc.scalar.activation(
                out=e, in_=ax, func=mybir.ActivationFunctionType.Exp, scale=-1.0
            )
            nc.scalar.activation(
                out=l1p, in_=e, func=mybir.ActivationFunctionType.Ln, bias=1.0
            )
            nc.vector.tensor_scalar_max(relux, x, 0.0)
            nc.vector.tensor_tensor(out=pk[:, 0:1], in0=relux, in1=l1p, op=ADD)

            # MSE term with float16 semantics.
            s16a = fin.tile([64, B], f16, tag="s16a")
            s16b = fin.tile([64, B], f16, tag="s16b")
            d16 = fin.tile([64, B], f16, tag="d16")
            sq16 = fin.tile([64, B], f16, tag="sq16")
            nc.vector.tensor_copy(out=s16a, in_=s_s)
            nc.vector.tensor_copy(out=s16b, in_=s_t)
            nc.vector.tensor_tensor(out=d16, in0=s16a, in1=s16b, op=SUB)
            nc.vector.tensor_tensor(out=sq16, in0=d16, in1=d16, op=MULT)
            nc.vector.reduce_sum(pk[:, 1:2], sq16, axis=X)

            # Partition-dim sums: relayout [64, 2] -> [1, 128] via DMA, then
            # one segmented reduce.
            flat = fin.tile([1, 128], f32, tag="flat")
            nc.gpsimd.dma_start(out=flat.rearrange("a (p c) -> a p c", c=2), in_=pk)
            red2 = fin.tile([1, 2], f32, tag="red2")
            nc.vector.reduce_sum(
                red2, flat.rearrange("a (p c) -> a c p", c=2), axis=X
            )

            contr = fin.tile([1, 1], f32, tag="contr")
            mse32 = fin.tile([1, 1], f32, tag="mse32")
            mse16 = fin.tile([1, 1], f16, tag="mse16")
            mseb = fin.tile([1, 1], f32, tag="mseb")
            lossv = fin.tile([1, 1], f32, tag="lossv")
            nc.vector.tensor_scalar_mul(contr, red2[:, 0:1], 1.0 / B)
            nc.vector.tensor_scalar_mul(mse32, red2[:, 1:2], 1.0 / (B * B))
            nc.vector.tensor_copy(out=mse16, in_=mse32)
            nc.vector.tensor_copy(out=mseb, in_=mse16)
            nc.vector.scalar_tensor_tensor(
                out=lossv, in0=mseb, scalar=0.3, in1=contr, op0=MULT, op1=ADD
            )
            nc.sync.dma_start(out=loss_d.ap(), in_=lossv)

    nc.compile()
    return nc


def get_nc():
    if "nc" not in _CACHE:
        _CACHE["nc"] = _build_nc()
    return _CACHE["nc"]


def _prep_q(q):
    # [B, N, D] fp32 -> [D, B*N] bf16 with n-major token order
    qn = np.ascontiguousarray(q.transpose(1, 0, 2).reshape(B * N, D))
    return np.ascontiguousarray(qn.T).astype(_BF16)


def _prep_d_shard(d, r):
    rows = d.reshape(C * S, D)[r * CL * S : (r + 1) * CL * S]
    return np.ascontiguousarray(rows.T).astype(_BF16)


def make_in_maps(query_embeddings, doc_embeddings, teacher_query_outputs,
                 teacher_doc_outputs):
    q = np.asarray(query_embeddings, dtype=np.float32)
    d = np.asarray(doc_embeddings, dtype=np.float32)
    tq = np.asarray(teacher_query_outputs, dtype=np.float32)
    td = np.asarray(teacher_doc_outputs, dtype=np.float32)
    qT = _prep_q(q)
    tqT = _prep_q(tq)
    eye = np.eye(B, dtype=np.float32)
    neye = (np.eye(B, dtype=np.float32) * np.float32(-1e30)).astype(np.float32)
    in_maps = []
    for r in range(NCORES):
        in_maps.append(
            {
                "qT": qT,
                "tqT": tqT,
                "dT": _prep_d_shard(d, r),
                "tdT": _prep_d_shard(td, r),
                "eye": eye,
                "neye": neye,
            }
        )
    return in_maps


def run(in_maps, trace=False):
    from concourse.bass_utils import run_bass_kernel_spmd

    nc = get_nc()
    return run_bass_kernel_spmd(
        nc, in_maps, core_ids=list(range(NCORES)), trace=trace
    )


def kernel(**inputs):
    in_maps = make_in_maps(**inputs)
    res = run(in_maps, trace=False)
    loss = np.asarray(res.results[0]["loss"], dtype=np.float32)
    return loss.reshape(())


# revision 14
# speedup vs baseline: 1.0679x; 1.0679x over previous
"""ColBERT pairwise-distill loss on 8 Trainium2 NeuronCores (Bass/Tile).

Strategy (doc-shard): each core holds all queries plus a 1/8 shard of the
docs (8 docs x 1024 tokens), computes local MaxSim scores [64, 8] for the
student and teacher streams, all-gathers score columns across the 8 cores,
and every core computes the final scalar loss (output read from core 0).

Per-core compute pipeline, per (tensor, query-chunk) unit:
  - 16 bf16 matmuls (K=128, M=128 q-tokens, N=512 doc-tokens) fill PSUM
    with fp32 similarities, 2 docs (4 banks) per wave.
  - wave 0 (docs 0-1): DVE reduce_max directly from PSUM (fp32).
  - waves 1-3 (docs 2-7): ScalarE evacuates PSUM -> SBUF bf16; VectorE
    folds a pairwise max tree at 2x bf16 rate, then a small tail reduce.
  - query tokens are laid out n-major so the sum over the 32 query tokens
    becomes two tiny strided reduce_sums plus one add.
"""

import numpy as np
import ml_dtypes

_BF16 = ml_dtypes.bfloat16

B, N, D, S, C = 64, 32, 128, 1024, 64
NCORES = 8
CL = C // NCORES          # docs per core
QCH = (B * N) // 128      # query-token chunks of 128
A_DOCS = 2                # docs per unit reduced directly from PSUM (fp32)
B_DOCS = CL - A_DOCS      # docs per unit via ACT-evac + bf16 max tree

_CACHE = {}


def _build_nc():
    import concourse.tile as tile
    from concourse import bacc, mybir

    f32 = mybir.dt.float32
    bf16 = mybir.dt.bfloat16
    f16 = mybir.dt.float16
    X = mybir.AxisListType.X
    MAX = mybir.AluOpType.max
    ADD = mybir.AluOpType.add
    SUB = mybir.AluOpType.subtract
    MULT = mybir.AluOpType.mult

    nc = bacc.Bacc(
        "TRN2", target_bir_lowering=False, debug=False, num_devices=NCORES
    )
    qT_d = nc.dram_tensor("qT", [128, B * N], bf16, kind="ExternalInput")
    tqT_d = nc.dram_tensor("tqT", [128, B * N], bf16, kind="ExternalInput")
    dT_d = nc.dram_tensor("dT", [128, CL * S], bf16, kind="ExternalInput")
    tdT_d = nc.dram_tensor("tdT", [128, CL * S], bf16, kind="ExternalInput")
    eye_d = nc.dram_tensor("eye", [B, B], f32, kind="ExternalInput")
    neye_d = nc.dram_tensor("neye", [B, B], f32, kind="ExternalInput")
    loss_d = nc.dram_tensor("loss", [1, 1], f32, kind="ExternalOutput")

    with tile.TileContext(nc) as tc:
        with (
            tc.tile_pool(name="inp", bufs=1) as inp,
            tc.tile_pool(name="ps", bufs=2, space="PSUM") as psp,
            tc.tile_pool(name="slab", bufs=4) as slabp,
            tc.tile_pool(name="tree", bufs=4) as treep,
            tc.tile_pool(name="fin", bufs=1) as fin,
            tc.tile_pool(name="dram", bufs=1, space="DRAM") as drp,
        ):
            qTs = inp.tile([128, B * N], bf16, tag="qT")
            tqTs = inp.tile([128, B * N], bf16, tag="tqT")
            dTs = inp.tile([128, CL * S], bf16, tag="dT")
            tdTs = inp.tile([128, CL * S], bf16, tag="tdT")
            nc.sync.dma_start(out=qTs, in_=qT_d.ap())
            nc.sync.dma_start(out=tqTs, in_=tqT_d.ap())
            nc.sync.dma_start(out=dTs, in_=dT_d.ap())
            nc.sync.dma_start(out=tdTs, in_=tdT_d.ap())

            eye_s = fin.tile([B, B], f32, tag="eye")
            neye_s = fin.tile([B, B], f32, tag="neye")
            nc.sync.dma_start(out=eye_s, in_=eye_d.ap())
            nc.sync.dma_start(out=neye_s, in_=neye_d.ap())

            # Load the exp/ln activation tables up front so they do not
            # stall the loss stage at the end.
            warm = fin.tile([64, 1], f32, tag="warm")
            nc.vector.memset(warm, 1.0)
            nc.scalar.activation(
                out=warm, in_=warm, func=mybir.ActivationFunctionType.Exp
            )
            nc.scalar.activation(
                out=warm, in_=warm, func=mybir.ActivationFunctionType.Ln
            )

            # buf[t][p, j, c]: per-chunk per-doc max-sums (fp32)
            buf = [
                fin.tile([128, QCH, CL], f32, name=f"buf{t}", tag=f"buf{t}")
                for t in range(2)
            ]
            # local scores [b, t, c], per-tensor DRAM bounce + gather targets
            sc = fin.tile([64, 2, CL], f32, tag="sc")
            scb = [
                drp.tile([64, CL], f32, name=f"scb{t}", tag=f"scb{t}")
                for t in range(2)
            ]
            gat = [
                drp.tile([NCORES, 64, CL], f32, name=f"gat{t}", tag=f"gat{t}")
                for t in range(2)
            ]

            for t, (qq, dd_) in enumerate([(qTs, dTs), (tqTs, tdTs)]):
                for j in range(QCH):
                    lhsT = qq[:, j * 128 : (j + 1) * 128]
                    slab = slabp.tile([128, B_DOCS, S], bf16, tag="slab")
                    for w in range(CL // 2):
                        pt = psp.tile([128, 2, S], f32, tag="pt")
                        for dd in range(2):
                            base = (w * 2 + dd) * S
                            for h in range(2):
                                nc.tensor.matmul(
                                    pt[:, dd, h * 512 : (h + 1) * 512],
                                    lhsT,
                                    dd_[:, base + h * 512 : base + (h + 1) * 512],
                                    start=True,
                                    stop=True,
                                )
                        if w == 0:
                            nc.vector.reduce_max(
                                buf[t][:, j, 0:A_DOCS], pt, axis=X
                            )
                        else:
                            nc.scalar.copy(
                                out=slab[:, (w - 1) * 2 : (w - 1) * 2 + 2, :],
                                in_=pt,
                            )
                    # bf16 pairwise max tree: 1024 -> 64, then tail reduce
                    t1 = treep.tile([128, B_DOCS, 512], bf16, tag="t1")
                    nc.vector.tensor_tensor(
                        out=t1, in0=slab[:, :, 0:512], in1=slab[:, :, 512:1024],
                        op=MAX,
                    )
                    t2 = treep.tile([128, B_DOCS, 256], bf16, tag="t2")
                    nc.vector.tensor_tensor(
                        out=t2, in0=t1[:, :, 0:256], in1=t1[:, :, 256:512], op=MAX
                    )
                    t3 = treep.tile([128, B_DOCS, 128], bf16, tag="t3")
                    nc.vector.tensor_tensor(
                        out=t3, in0=t2[:, :, 0:128], in1=t2[:, :, 128:256], op=MAX
                    )
                    t4 = treep.tile([128, B_DOCS, 64], bf16, tag="t4")
                    nc.vector.tensor_tensor(
                        out=t4, in0=t3[:, :, 0:64], in1=t3[:, :, 64:128], op=MAX
                    )
                    nc.vector.reduce_max(buf[t][:, j, A_DOCS:CL], t4, axis=X)
                # assemble this tensor's local score columns and start its
                # all-gather while the other tensor still computes
                h0 = fin.tile([64, CL], f32, name=f"h0_{t}", tag=f"h0_{t}")
                h1 = fin.tile([64, CL], f32, name=f"h1_{t}", tag=f"h1_{t}")
                nc.vector.reduce_sum(
                    h0, buf[t][0:64].rearrange("p j c -> p c j"), axis=X
                )
                nc.vector.reduce_sum(
                    h1, buf[t][64:128].rearrange("p j c -> p c j"), axis=X
                )
                nc.vector.tensor_tensor(out=sc[:, t, :], in0=h0, in1=h1, op=ADD)
                nc.sync.dma_start(out=scb[t], in_=sc[:, t, :])
                nc.gpsimd.collective_compute(
                    "AllGather",
                    mybir.AluOpType.bypass,
                    replica_groups=[list(range(NCORES))],
                    ins=[scb[t][:, :]],
                    outs=[gat[t][:, :, :]],
                )

            # Load gathered score columns: scores[b, r*CL + c].
            scf = fin.tile([64, 2, NCORES * CL], f32, tag="scf")
            for t in range(2):
                nc.sync.dma_start(
                    out=scf[:, t, :].rearrange("b (r c) -> b r c", r=NCORES),
                    in_=gat[t].rearrange("r b c -> b r c"),
                )
            s_s = scf[:, 0, :]
            s_t = scf[:, 1, :]

            # Contrastive term.
            tmp = fin.tile([64, B], f32, tag="tmp")
            pos = fin.tile([64, 1], f32, tag="pos")
            neg = fin.tile([64, 1], f32, tag="neg")
            x = fin.tile([64, 1], f32, tag="x")
            ax = fin.tile([64, 1], f32, tag="ax")
            e = fin.tile([64, 1], f32, tag="e")
            l1p = fin.tile([64, 1], f32, tag="l1p")
            relux = fin.tile([64, 1], f32, tag="relux")
            pk = fin.tile([64, 2], f32, tag="pk")

            nc.vector.tensor_tensor(out=tmp, in0=s_s, in1=eye_s, op=MULT)
            nc.vector.reduce_sum(pos, tmp, axis=X)
            nc.vector.tensor_tensor(out=tmp, in0=s_s, in1=neye_s, op=ADD)
            nc.vector.reduce_max(neg, tmp, axis=X)
            nc.vector.tensor_tensor(out=x, in0=neg, in1=pos, op=SUB)
            # softplus(x) = max(x, 0) + ln(1 + exp(-|x|))
            nc.vector.scalar_tensor_tensor(
                out=ax, in0=x, scalar=-1.0, in1=x, op0=MULT, op1=MAX
            )
            nc.scalar.activation(
                out=e, in_=ax, func=mybir.ActivationFunctionType.Exp, scale=-1.0
            )
            nc.scalar.activation(
                out=l1p, in_=e, func=mybir.ActivationFunctionType.Ln, bias=1.0
            )
            nc.vector.tensor_scalar_max(relux, x, 0.0)
            nc.vector.tensor_tensor(out=pk[:, 0:1], in0=relux, in1=l1p, op=ADD)

            # MSE term with float16 semantics.
            s16a = fin.tile([64, B], f16, tag="s16a")
            s16b = fin.tile([64, B], f16, tag="s16b")
            d16 = fin.tile([64, B], f16, tag="d16")
            sq16 = fin.tile([64, B], f16, tag="sq16")
            nc.vector.tensor_copy(out=s16a, in_=s_s)
            nc.vector.tensor_copy(out=s16b, in_=s_t)
            nc.vector.tensor_tensor(out=d16, in0=s16a, in1=s16b, op=SUB)
            nc.vector.tensor_tensor(out=sq16, in0=d16, in1=d16, op=MULT)
            nc.vector.reduce_sum(pk[:, 1:2], sq16, axis=X)

            # Partition-dim sums: relayout [64, 2] -> [1, 128] via DMA, then
            # one segmented reduce.
            flat = fin.tile([1, 128], f32, tag="flat")
            nc.gpsimd.dma_start(out=flat.rearrange("a (p c) -> a p c", c=2), in_=pk)
            red2 = fin.tile([1, 2], f32, tag="red2")
            nc.vector.reduce_sum(
                red2, flat.rearrange("a (p c) -> a c p", c=2), axis=X
            )

            contr = fin.tile([1, 1], f32, tag="contr")
            mse32 = fin.tile([1, 1], f32, tag="mse32")
            mse16 = fin.tile([1, 1], f16, tag="mse16")
            mseb = fin.tile([1, 1], f32, tag="mseb")
            lossv = fin.tile([1, 1], f32, tag="lossv")
            nc.vector.tensor_scalar_mul(contr, red2[:, 0:1], 1.0 / B)
            nc.vector.tensor_scalar_mul(mse32, red2[:, 1:2], 1.0 / (B * B))
            nc.vector.tensor_copy(out=mse16, in_=mse32)
            nc.vector.tensor_copy(out=mseb, in_=mse16)
            nc.vector.scalar_tensor_tensor(
                out=lossv, in0=mseb, scalar=0.3, in1=contr, op0=MULT, op1=ADD
            )
            nc.sync.dma_start(out=loss_d.ap(), in_=lossv)

    nc.compile()
    return nc


def get_nc():
    if "nc" not in _CACHE:
        _CACHE["nc"] = _build_nc()
    return _CACHE["nc"]


def _prep_q(q):
    # [B, N, D] fp32 -> [D, B*N] bf16 with n-major token order
    qn = np.ascontiguousarray(q.transpose(1, 0, 2).reshape(B * N, D))
    return np.ascontiguousarray(qn.T).astype(_BF16)


def _prep_d_shard(d, r):
    rows = d.reshape(C * S, D)[r * CL * S : (r + 1) * CL * S]
    return np.ascontiguousarray(rows.T).astype(_BF16)


def make_in_maps(query_embeddings, doc_embeddings, teacher_query_outputs,
                 teacher_doc_outputs):
    q = np.asarray(query_embeddings, dtype=np.float32)
    d = np.asarray(doc_embeddings, dtype=np.float32)
    tq = np.asarray(teacher_query_outputs, dtype=np.float32)
    td = np.asarray(teacher_doc_outputs, dtype=np.float32)
    qT = _prep_q(q)
    tqT = _prep_q(tq)
    eye = np.eye(B, dtype=np.float32)
    neye = (np.eye(B, dtype=np.float32) * np.float32(-1e30)).astype(np.float32)
    in_maps = []
    for r in range(NCORES):
        in_maps.append(
            {
                "qT": qT,
                "tqT": tqT,
                "dT": _prep_d_shard(d, r),
                "tdT": _prep_d_shard(td, r),
                "eye": eye,
                "neye": neye,
            }
        )
    return in_maps


def run(in_maps, trace=False):
    from concourse.bass_utils import run_bass_kernel_spmd

    nc = get_nc()
    return run_bass_kernel_spmd(
        nc, in_maps, core_ids=list(range(NCORES)), trace=trace
    )


def kernel(**inputs):
    in_maps = make_in_maps(**inputs)
    res = run(in_maps, trace=False)
    loss = np.asarray(res.results[0]["loss"], dtype=np.float32)
    return loss.reshape(())
